# revision 19
# baseline (speedup 1.0000x reference)
"""Trainium2 Bass kernel for the DeformableCurrents loss.

Energy e = e_ss - 2*e_st + e_tt where e_xy = sum_ij K(c_i, c_j) * <n_i, n_j>
with the Cauchy kernel K = 1/(1 + |ci - cj|^2).

Strategy (8-core SPMD, identical instruction stream per core, per-core data
staged by the host):
  - P-matmul (K=5 float32r):  P[j, i] = 1 + |y_j - x_i|^2 via augmented
    features, lhsT = feature block of 128 "j" points, rhs = feature chunk of
    512 "i" points -> PSUM [128, 512].
  - reciprocal: 3 of 4 units per group via DVE custom fast-reciprocal
    ([128,1536] in one op), 1 unit via ACT exp(-ln P). Output bf16.
  - S-matmul (K=128, M=3, bf16): S[d, i] += sum_j w*m[d,j] * Pinv[j,i],
    accumulated in PSUM over the 4 units of a pseudo-group. The symmetric
    doubling weight (and the -2 for e_st) is baked into the normals.
  - ACT copies S tiles out of PSUM; host computes sum_d,i n[d,i]*S[d,i].

Work decomposition: i-chunks of 512, j-blocks of 128. For the symmetric ss/tt
matrices only diagonal 512x512 super-blocks (weight 1) and strictly-upper
blocks (weight 2) are computed. Total units 2112 = 8 cores x 66 groups x 4.
"""

import numpy as np

V, N, M = 4096, 8192, 8192
CHUNK = 512
BLOCK = 128
NCORES = 8
PGS_PER_CORE = 66
UNITS_PER_PG = 4

_CACHED_NC = None


# ---------------------------------------------------------------- planning
def _plan():
    """Global ordered list of 528 pseudo-groups (matrix, chunk, blocks[4], w[4])."""
    pgs = []
    for m in ("ss", "tt", "st"):
        for c in range(16):
            if m == "st":
                blocks = [(b, -2.0) for b in range(64)]
            else:
                blocks = [(b, 1.0) for b in range(4 * c, 4 * c + 4)]
                blocks += [(b, 2.0) for b in range(4 * c + 4, 64)]
            for k in range(0, len(blocks), 4):
                quad = blocks[k : k + 4]
                pgs.append((m, c, [b for b, _ in quad], [w for _, w in quad]))
    assert len(pgs) == NCORES * PGS_PER_CORE
    return pgs


# ---------------------------------------------------------------- bass build
def _build_nc():
    global _CACHED_NC
    if _CACHED_NC is not None:
        return _CACHED_NC

    from contextlib import ExitStack

    import concourse.bass as bass
    import concourse.tile as tile
    from concourse import bacc, mybir
    from concourse.dve_ops import RECIP_APPROX_FAST_CONSTS, RECIPROCAL_APPROX_FAST

    F32 = mybir.dt.float32
    F32R = mybir.dt.float32r
    BF16 = mybir.dt.bfloat16
    AF = mybir.ActivationFunctionType

    nc = bacc.Bacc("TRN2", target_bir_lowering=False, debug=False,
                   num_devices=NCORES)

    # Pin Ln/Exp/Copy to the one table set that contains all three, so the
    # table-load fixpoint emits a single LoadActFuncSet instead of swapping
    # sets around every ln->exp->copy sequence (~2.5us per swap).
    from concourse.hw_specs import get_activation_tables
    _tabs = get_activation_tables(nc.m.arch)
    _pinned = {AF.Ln, AF.Exp, AF.Copy}
    if "natural_log_exp_and_others" in _tabs:
        for _name, _fns in _tabs.items():
            if _name != "natural_log_exp_and_others":
                _fns -= _pinned

    # feature slabs laid out feature-row-major so a 6-pg slice is one
    # clean 3D access pattern: [5, 66, 512] / [128, 66, 12]
    wfeat_d = nc.dram_tensor("wfeat", [5, PGS_PER_CORE, 512], F32R,
                             kind="ExternalInput").ap()
    rhsf_d = nc.dram_tensor("rhsf", [5, PGS_PER_CORE, 512], F32R,
                            kind="ExternalInput").ap()
    wnrm_d = nc.dram_tensor("wnrm", [128, PGS_PER_CORE, 12], BF16,
                            kind="ExternalInput").ap()
    # S results packed at 32-aligned partition bases {0,32,64,96} x 17
    # column blocks so the final DMA is wide
    sout_d = nc.dram_tensor("sout", [99, 17 * 512], F32,
                            kind="ExternalOutput").ap()

    rc = RECIP_APPROX_FAST_CONSTS

    with tile.TileContext(nc) as tc, ExitStack() as ctx:
        stage = ctx.enter_context(tc.tile_pool(name="stage", bufs=3))
        lnp = ctx.enter_context(tc.tile_pool(name="lnp", bufs=2))
        piv = ctx.enter_context(tc.tile_pool(name="piv", bufs=2))
        outp = ctx.enter_context(tc.tile_pool(name="outp", bufs=1))
        dvePA = ctx.enter_context(
            tc.tile_pool(name="dvePA", bufs=2, space=bass.MemorySpace.PSUM))
        dvePB = ctx.enter_context(
            tc.tile_pool(name="dvePB", bufs=1, space=bass.MemorySpace.PSUM))
        actP = ctx.enter_context(
            tc.tile_pool(name="actP", bufs=1, space=bass.MemorySpace.PSUM))
        sP = ctx.enter_context(
            tc.tile_pool(name="sP", bufs=2, space=bass.MemorySpace.PSUM))

        sout = outp.tile([99, 17 * 512], F32, tag="sout")

        prev = None       # (pia, pidB, pidA, wnrm_s, p) of previous pg
        pending = []      # [(s3_t, p)] egresses delayed by one more slot

        def emit_mms(prev):
            # S matmuls of the previous pg (PE stream, after this pg's MMPs)
            pia, pidB, pidA, wnrm_s, p = prev
            s3_t = sP.tile([3, 512], F32, tag="s3")
            nc.tensor.matmul(s3_t[:], wnrm_s[:, 0:3], pia[:],
                             start=True, stop=False)
            nc.tensor.matmul(s3_t[:], wnrm_s[:, 3:6], pidB[:],
                             start=False, stop=False)
            for k in range(2):
                nc.tensor.matmul(s3_t[:], wnrm_s[:, 3 * (k + 2) : 3 * (k + 3)],
                                 pidA[:, 512 * k : 512 * (k + 1)],
                                 start=False, stop=(k == 1))
            return s3_t

        def emit_egress(s3_t, p):
            r, cblk = p % 4, p // 4
            nc.scalar.activation(
                sout[32 * r : 32 * r + 3, 512 * cblk : 512 * (cblk + 1)],
                s3_t[:], AF.Copy)

        SGB = 6  # pgs per staged DMA batch
        for p in range(PGS_PER_CORE):
            if p % SGB == 0:
                wfeat_t = stage.tile([5, SGB, 512], F32R, tag="wfeat")
                nc.sync.dma_start(wfeat_t[:], wfeat_d[:, p : p + SGB, :])
                rhsf_t = stage.tile([5, SGB, 512], F32R, tag="rhsf")
                nc.gpsimd.dma_start(rhsf_t[:], rhsf_d[:, p : p + SGB, :])
                wnrm_t = stage.tile([128, SGB, 12], BF16, tag="wnrm")
                nc.gpsimd.dma_start(wnrm_t[:], wnrm_d[:, p : p + SGB, :])
            s = p % SGB
            wfeat_s = wfeat_t[:, s, :]
            rhsf_s = rhsf_t[:, s, :]
            wnrm_s = wnrm_t[:, s, :]

            # ---- P matmuls: u0 -> actP, u1 -> dvePB, u2/u3 -> dvePA halves
            act_ps = actP.tile([128, 512], F32, tag="actps")
            nc.tensor.matmul(act_ps[:], wfeat_s[:, 0:128], rhsf_s[:],
                             start=True, stop=True)
            dve_psB = dvePB.tile([128, 512], F32, tag="dvepsB")
            nc.tensor.matmul(dve_psB[:], wfeat_s[:, 128:256], rhsf_s[:],
                             start=True, stop=True)
            dve_psA = dvePA.tile([128, 1024], F32, tag="dvepsA")
            for k in range(2):
                nc.tensor.matmul(dve_psA[:, 512 * k : 512 * (k + 1)],
                                 wfeat_s[:, 128 * (k + 2) : 128 * (k + 3)],
                                 rhsf_s[:], start=True, stop=True)

            # ---- reciprocals
            lnb = lnp.tile([128, 512], F32, tag="lnb")
            nc.scalar.activation(lnb[:], act_ps[:], AF.Ln)
            pia = piv.tile([128, 512], BF16, tag="pia")
            nc.scalar.activation(pia[:], lnb[:], AF.Exp, scale=-1.0)
            pidB = piv.tile([128, 512], BF16, tag="pidB")
            nc.vector._custom_dve(RECIPROCAL_APPROX_FAST, out=pidB[:],
                                  in0=dve_psB[:], s0=rc["s0"], s1=rc["s1"],
                                  imm2=rc["imm2"])
            pidA = piv.tile([128, 1024], BF16, tag="pidA")
            nc.vector._custom_dve(RECIPROCAL_APPROX_FAST, out=pidA[:],
                                  in0=dve_psA[:], s0=rc["s0"], s1=rc["s1"],
                                  imm2=rc["imm2"])

            # ---- previous pg's S matmuls follow this pg's P matmuls in the
            # PE stream (PE never waits on this pg's reciprocals); egresses
            # are delayed one further slot so ACT never waits on MMS
            if prev is not None:
                pending.append((emit_mms(prev), prev[4]))
            if len(pending) > 1:
                emit_egress(*pending.pop(0))

            prev = (pia, pidB, pidA, wnrm_s, p)

        pending.append((emit_mms(prev), prev[4]))
        for item in pending:
            emit_egress(*item)
        nc.sync.dma_start(sout_d[:], sout[:])

    nc.compile()
    _CACHED_NC = nc
    return nc


# ---------------------------------------------------------------- host side
def _feats(pts):
    """pts [n,3] f32 -> featL [5,n] (lhsT side), featR [5,n] (rhs side)."""
    x, y, z = pts[:, 0], pts[:, 1], pts[:, 2]
    n2 = x * x + y * y + z * z
    one = np.ones_like(n2)
    featL = np.stack([x, y, z, n2, one]).astype(np.float32)
    featR = np.stack([-2 * x, -2 * y, -2 * z, one, n2 + 1.0]).astype(np.float32)
    return featL, featR


def kernel(src_vertices, tar_normals, tar_centers, src_indices):
    import ml_dtypes
    from concourse.bass_utils import run_bass_kernel_spmd

    src_vertices = np.asarray(src_vertices, dtype=np.float32)
    tar_normals = np.asarray(tar_normals, dtype=np.float32)
    tar_centers = np.asarray(tar_centers, dtype=np.float32)
    idx = np.asarray(src_indices).astype(np.int64)

    # triangle gather: normals and centers of source triangles
    tris = src_vertices[idx]                      # [N, 3, 3]
    a, b, c = tris[:, 0, :], tris[:, 1, :], tris[:, 2, :]
    normals = 0.5 * np.cross(a - b, c - b).astype(np.float32)   # [N,3]
    centers = (tris.sum(axis=1) / 3.0).astype(np.float32)       # [N,3]

    sfL, sfR = _feats(centers)
    tfL, tfR = _feats(tar_centers)
    snT = normals.T.astype(np.float64)        # [3, N] finalize side
    tnT = tar_normals.T.astype(np.float64)

    featL = {"ss": sfL, "tt": tfL, "st": tfL}   # partition (j) side
    featR = {"ss": sfR, "tt": tfR, "st": sfR}   # free (i) side
    nrmP = {"ss": normals, "tt": tar_normals, "st": tar_normals}  # [n,3] j side
    fnT = {"ss": snT, "tt": tnT, "st": snT}     # [3,n] i side (host)

    pgs = _plan()
    in_maps = []
    fn_slices = []  # per core, per pg: [3,512] f64 host-side finalize normals
    for core in range(NCORES):
        my = pgs[core * PGS_PER_CORE : (core + 1) * PGS_PER_CORE]
        wfeat = np.empty((PGS_PER_CORE, 5, 512), np.float32)
        rhsf = np.empty((PGS_PER_CORE, 5, 512), np.float32)
        wnrm = np.empty((PGS_PER_CORE, 128, 12), np.float32)
        fns = []
        for p, (m, cch, blocks, ws) in enumerate(my):
            rhsf[p] = featR[m][:, CHUNK * cch : CHUNK * (cch + 1)]
            for q, (blk, w) in enumerate(zip(blocks, ws)):
                wfeat[p, :, 128 * q : 128 * (q + 1)] = (
                    featL[m][:, BLOCK * blk : BLOCK * (blk + 1)])
                wnrm[p, :, 3 * q : 3 * (q + 1)] = (
                    w * nrmP[m][BLOCK * blk : BLOCK * (blk + 1), :])
            fns.append(fnT[m][:, CHUNK * cch : CHUNK * (cch + 1)])
        in_maps.append({
            "wfeat": np.ascontiguousarray(wfeat.transpose(1, 0, 2)),
            "rhsf": np.ascontiguousarray(rhsf.transpose(1, 0, 2)),
            "wnrm": np.ascontiguousarray(
                wnrm.transpose(1, 0, 2)).astype(ml_dtypes.bfloat16),
        })
        fn_slices.append(fns)

    nc = _build_nc()
    results = run_bass_kernel_spmd(nc, in_maps, list(range(NCORES))).results

    e = 0.0
    for core in range(NCORES):
        sout = np.asarray(results[core]["sout"], dtype=np.float64)  # [99, 17*512]
        for p in range(PGS_PER_CORE):
            r, cblk = p % 4, p // 4
            S = sout[32 * r : 32 * r + 3, 512 * cblk : 512 * (cblk + 1)]
            e += float((S * fn_slices[core][p]).sum())
    return np.float32(e)


# revision 20
# speedup vs baseline: 2.2236x; 2.2236x over previous
"""Trainium2 Bass kernel for the DeformableCurrents loss.

Energy e = e_ss - 2*e_st + e_tt where e_xy = sum_ij K(c_i, c_j) * <n_i, n_j>
with the Cauchy kernel K = 1/(1 + |ci - cj|^2).

Strategy (8-core SPMD, identical instruction stream per core, per-core data
staged by the host):
  - P-matmul (K=5 float32r):  P[j, i] = 1 + |y_j - x_i|^2 via augmented
    features, lhsT = feature block of 128 "j" points, rhs = feature chunk of
    512 "i" points -> PSUM [128, 512].
  - reciprocal: 3 of 4 units per group via DVE custom fast-reciprocal
    ([128,1536] in one op), 1 unit via ACT exp(-ln P). Output bf16.
  - S-matmul (K=128, M=3, bf16): S[d, i] += sum_j w*m[d,j] * Pinv[j,i],
    accumulated in PSUM over the 4 units of a pseudo-group. The symmetric
    doubling weight (and the -2 for e_st) is baked into the normals.
  - ACT copies S tiles out of PSUM; host computes sum_d,i n[d,i]*S[d,i].

Work decomposition: i-chunks of 512, j-blocks of 128. For the symmetric ss/tt
matrices only diagonal 512x512 super-blocks (weight 1) and strictly-upper
blocks (weight 2) are computed. Total units 2112 = 8 cores x 66 groups x 4.
"""

import numpy as np

V, N, M = 4096, 8192, 8192
CHUNK = 512
BLOCK = 128
NCORES = 8
PGS_PER_CORE = 66
UNITS_PER_PG = 4
_ACTIVE_PGS = None  # test hook: if set, only this many pgs are emitted

_CACHED_NC = None


# ---------------------------------------------------------------- planning
def _plan():
    """Global ordered list of 528 pseudo-groups (matrix, chunk, blocks[4], w[4])."""
    pgs = []
    for m in ("ss", "tt", "st"):
        for c in range(16):
            if m == "st":
                blocks = [(b, -2.0) for b in range(64)]
            else:
                blocks = [(b, 1.0) for b in range(4 * c, 4 * c + 4)]
                blocks += [(b, 2.0) for b in range(4 * c + 4, 64)]
            for k in range(0, len(blocks), 4):
                quad = blocks[k : k + 4]
                pgs.append((m, c, [b for b, _ in quad], [w for _, w in quad]))
    assert len(pgs) == NCORES * PGS_PER_CORE
    return pgs


# ---------------------------------------------------------------- bass build
def _build_nc():
    global _CACHED_NC
    if _CACHED_NC is not None:
        return _CACHED_NC

    from contextlib import ExitStack

    import concourse.bass as bass
    import concourse.tile as tile
    from concourse import bacc, mybir
    from concourse.dve_ops import RECIP_APPROX_FAST_CONSTS, RECIPROCAL_APPROX_FAST

    F32 = mybir.dt.float32
    F32R = mybir.dt.float32r
    BF16 = mybir.dt.bfloat16
    AF = mybir.ActivationFunctionType

    nc = bacc.Bacc("TRN2", target_bir_lowering=False, debug=False,
                   num_devices=NCORES)

    # Pin Ln/Exp/Copy to the one table set that contains all three, so the
    # table-load fixpoint emits a single LoadActFuncSet instead of swapping
    # sets around every ln->exp->copy sequence (~2.5us per swap).
    from concourse.hw_specs import get_activation_tables
    _tabs = get_activation_tables(nc.m.arch)
    _pinned = {AF.Ln, AF.Exp, AF.Copy}
    if "natural_log_exp_and_others" in _tabs:
        for _name, _fns in _tabs.items():
            if _name != "natural_log_exp_and_others":
                _fns -= _pinned

    # feature slabs laid out feature-row-major so a 6-pg slice is one
    # clean 3D access pattern: [5, 66, 512] / [128, 66, 12]
    wfeat_d = nc.dram_tensor("wfeat", [5, PGS_PER_CORE, 512], F32R,
                             kind="ExternalInput").ap()
    rhsf_d = nc.dram_tensor("rhsf", [5, PGS_PER_CORE, 512], F32R,
                            kind="ExternalInput").ap()
    wnrm_d = nc.dram_tensor("wnrm", [128, PGS_PER_CORE, 12], BF16,
                            kind="ExternalInput").ap()
    # S results packed at 32-aligned partition bases {0,32,64,96} x 17
    # column blocks so the final DMA is wide
    sout_d = nc.dram_tensor("sout", [99, 17 * 512], F32,
                            kind="ExternalOutput").ap()

    rc = RECIP_APPROX_FAST_CONSTS

    with tile.TileContext(nc) as tc, ExitStack() as ctx:
        stage = ctx.enter_context(tc.tile_pool(name="stage", bufs=3))
        lnp = ctx.enter_context(tc.tile_pool(name="lnp", bufs=2))
        piv = ctx.enter_context(tc.tile_pool(name="piv", bufs=2))
        outp = ctx.enter_context(tc.tile_pool(name="outp", bufs=1))
        dvePA = ctx.enter_context(
            tc.tile_pool(name="dvePA", bufs=2, space=bass.MemorySpace.PSUM))
        dvePB = ctx.enter_context(
            tc.tile_pool(name="dvePB", bufs=1, space=bass.MemorySpace.PSUM))
        actP = ctx.enter_context(
            tc.tile_pool(name="actP", bufs=1, space=bass.MemorySpace.PSUM))
        sP = ctx.enter_context(
            tc.tile_pool(name="sP", bufs=2, space=bass.MemorySpace.PSUM))

        sout = outp.tile([99, 17 * 512], F32, tag="sout")

        prev = None       # (pia, pidB, pidA, wnrm_s, p) of previous pg
        pending = []      # [(s3_t, p)] egresses delayed by one more slot

        def emit_mms(prev):
            # S matmuls of the previous pg (PE stream, after this pg's MMPs)
            pia, pidB, pidA, wnrm_s, p = prev
            s3_t = sP.tile([3, 512], F32, tag="s3")
            nc.tensor.matmul(s3_t[:], wnrm_s[:, 0:3], pia[:],
                             start=True, stop=False)
            nc.tensor.matmul(s3_t[:], wnrm_s[:, 3:6], pidB[:],
                             start=False, stop=False)
            for k in range(2):
                nc.tensor.matmul(s3_t[:], wnrm_s[:, 3 * (k + 2) : 3 * (k + 3)],
                                 pidA[:, 512 * k : 512 * (k + 1)],
                                 start=False, stop=(k == 1))
            return s3_t

        def emit_egress(s3_t, p):
            r, cblk = p % 4, p // 4
            nc.scalar.activation(
                sout[32 * r : 32 * r + 3, 512 * cblk : 512 * (cblk + 1)],
                s3_t[:], AF.Copy)

        SGB = 6  # pgs per staged DMA batch
        n_active = _ACTIVE_PGS if _ACTIVE_PGS is not None else PGS_PER_CORE
        for p in range(n_active):
            if p % SGB == 0:
                wfeat_t = stage.tile([5, SGB, 512], F32R, tag="wfeat")
                nc.sync.dma_start(wfeat_t[:], wfeat_d[:, p : p + SGB, :])
                rhsf_t = stage.tile([5, SGB, 512], F32R, tag="rhsf")
                nc.gpsimd.dma_start(rhsf_t[:], rhsf_d[:, p : p + SGB, :])
                wnrm_t = stage.tile([128, SGB, 12], BF16, tag="wnrm")
                nc.gpsimd.dma_start(wnrm_t[:], wnrm_d[:, p : p + SGB, :])
            s = p % SGB
            wfeat_s = wfeat_t[:, s, :]
            rhsf_s = rhsf_t[:, s, :]
            wnrm_s = wnrm_t[:, s, :]

            # ---- P matmuls: u0 -> actP, u1 -> dvePB, u2/u3 -> dvePA halves
            act_ps = actP.tile([128, 512], F32, tag="actps")
            nc.tensor.matmul(act_ps[:], wfeat_s[:, 0:128], rhsf_s[:],
                             start=True, stop=True)
            dve_psB = dvePB.tile([128, 512], F32, tag="dvepsB")
            nc.tensor.matmul(dve_psB[:], wfeat_s[:, 128:256], rhsf_s[:],
                             start=True, stop=True)
            dve_psA = dvePA.tile([128, 1024], F32, tag="dvepsA")
            for k in range(2):
                nc.tensor.matmul(dve_psA[:, 512 * k : 512 * (k + 1)],
                                 wfeat_s[:, 128 * (k + 2) : 128 * (k + 3)],
                                 rhsf_s[:], start=True, stop=True)

            # ---- reciprocals
            lnb = lnp.tile([128, 512], F32, tag="lnb")
            nc.scalar.activation(lnb[:], act_ps[:], AF.Ln)
            pia = piv.tile([128, 512], BF16, tag="pia")
            nc.scalar.activation(pia[:], lnb[:], AF.Exp, scale=-1.0)
            pidB = piv.tile([128, 512], BF16, tag="pidB")
            nc.vector._custom_dve(RECIPROCAL_APPROX_FAST, out=pidB[:],
                                  in0=dve_psB[:], s0=rc["s0"], s1=rc["s1"],
                                  imm2=rc["imm2"])
            pidA = piv.tile([128, 1024], BF16, tag="pidA")
            nc.vector._custom_dve(RECIPROCAL_APPROX_FAST, out=pidA[:],
                                  in0=dve_psA[:], s0=rc["s0"], s1=rc["s1"],
                                  imm2=rc["imm2"])

            # ---- previous pg's S matmuls follow this pg's P matmuls in the
            # PE stream (PE never waits on this pg's reciprocals); egresses
            # are delayed one further slot so ACT never waits on MMS
            if prev is not None:
                pending.append((emit_mms(prev), prev[4]))
            if len(pending) > 1:
                emit_egress(*pending.pop(0))

            prev = (pia, pidB, pidA, wnrm_s, p)

        pending.append((emit_mms(prev), prev[4]))
        for item in pending:
            emit_egress(*item)
        nc.sync.dma_start(sout_d[:], sout[:])

    nc.compile()
    _CACHED_NC = nc
    return nc


# ---------------------------------------------------------------- host side
def _feats(pts):
    """pts [n,3] f32 -> featL [5,n] (lhsT side), featR [5,n] (rhs side)."""
    x, y, z = pts[:, 0], pts[:, 1], pts[:, 2]
    n2 = x * x + y * y + z * z
    one = np.ones_like(n2)
    featL = np.stack([x, y, z, n2, one]).astype(np.float32)
    featR = np.stack([-2 * x, -2 * y, -2 * z, one, n2 + 1.0]).astype(np.float32)
    return featL, featR


def kernel(src_vertices, tar_normals, tar_centers, src_indices):
    import ml_dtypes
    from concourse.bass_utils import run_bass_kernel_spmd

    src_vertices = np.asarray(src_vertices, dtype=np.float32)
    tar_normals = np.asarray(tar_normals, dtype=np.float32)
    tar_centers = np.asarray(tar_centers, dtype=np.float32)
    idx = np.asarray(src_indices).astype(np.int64)

    # triangle gather: normals and centers of source triangles
    tris = src_vertices[idx]                      # [N, 3, 3]
    a, b, c = tris[:, 0, :], tris[:, 1, :], tris[:, 2, :]
    normals = 0.5 * np.cross(a - b, c - b).astype(np.float32)   # [N,3]
    centers = (tris.sum(axis=1) / 3.0).astype(np.float32)       # [N,3]

    sfL, sfR = _feats(centers)
    tfL, tfR = _feats(tar_centers)
    snT = normals.T.astype(np.float64)        # [3, N] finalize side
    tnT = tar_normals.T.astype(np.float64)

    featL = {"ss": sfL, "tt": tfL, "st": tfL}   # partition (j) side
    featR = {"ss": sfR, "tt": tfR, "st": sfR}   # free (i) side
    nrmP = {"ss": normals, "tt": tar_normals, "st": tar_normals}  # [n,3] j side
    fnT = {"ss": snT, "tt": tnT, "st": snT}     # [3,n] i side (host)

    pgs = _plan()
    in_maps = []
    fn_slices = []  # per core, per pg: [3,512] f64 host-side finalize normals
    for core in range(NCORES):
        my = pgs[core * PGS_PER_CORE : (core + 1) * PGS_PER_CORE]
        wfeat = np.empty((PGS_PER_CORE, 5, 512), np.float32)
        rhsf = np.empty((PGS_PER_CORE, 5, 512), np.float32)
        wnrm = np.empty((PGS_PER_CORE, 128, 12), np.float32)
        fns = []
        for p, (m, cch, blocks, ws) in enumerate(my):
            rhsf[p] = featR[m][:, CHUNK * cch : CHUNK * (cch + 1)]
            for q, (blk, w) in enumerate(zip(blocks, ws)):
                wfeat[p, :, 128 * q : 128 * (q + 1)] = (
                    featL[m][:, BLOCK * blk : BLOCK * (blk + 1)])
                wnrm[p, :, 3 * q : 3 * (q + 1)] = (
                    w * nrmP[m][BLOCK * blk : BLOCK * (blk + 1), :])
            fns.append(fnT[m][:, CHUNK * cch : CHUNK * (cch + 1)])
        in_maps.append({
            "wfeat": np.ascontiguousarray(wfeat.transpose(1, 0, 2)),
            "rhsf": np.ascontiguousarray(rhsf.transpose(1, 0, 2)),
            "wnrm": np.ascontiguousarray(
                wnrm.transpose(1, 0, 2)).astype(ml_dtypes.bfloat16),
        })
        fn_slices.append(fns)

    nc = _build_nc()
    results = run_bass_kernel_spmd(nc, in_maps, list(range(NCORES))).results

    e = 0.0
    for core in range(NCORES):
        sout = np.asarray(results[core]["sout"], dtype=np.float64)  # [99, 17*512]
        for p in range(PGS_PER_CORE):
            r, cblk = p % 4, p // 4
            S = sout[32 * r : 32 * r + 3, 512 * cblk : 512 * (cblk + 1)]
            e += float((S * fn_slices[core][p]).sum())
    return np.float32(e)
